# revision 22
# baseline (speedup 1.0000x reference)
"""Trainium2 Bass kernel for nn_CovBlock (B=4, N=8192, D=2048, H=512, F=64).

Data-parallel over 8 NeuronCores: x sharded along N (1024 rows/batch/core).
Per core: one streaming pass over its x shard computing per-column
sum-of-squares of row-centered x, accumulated per batch in PSUM via
TensorE matmuls with a one-hot stationary operand.  The row-mean is folded
into the Square activation (scale=D, bias=-rowsum) so ss is accumulated
scaled by D^2; the eps in cov = ss/(ss+eps) is rescaled to match.

Cross-core reduction of ss is split unevenly: batch 0's AllReduce fires at
~1/4 of the x stream and absorbs the first-collective cost (NCCL barrier +
inter-core launch skew, ~40us) entirely under the remaining stream;
batches 1-3 go through an AllGather of locally-transposed partials at the
end (AllGather has a ~2x lower latency floor than AllReduce; the 8-way sum
is done locally on VectorE).  The 3-layer MLP is column-sharded (W1) /
row-sharded (W2) across cores with a final small AllGather for the
layer-2 partial sums.

Engine discipline: x streams on the Sync HWDGE ring (2MB chunks,
contiguous 16KB-per-partition descriptors); small tail DMAs ride the
Scalar (ACT) HWDGE ring; GpSimd (SWDGE) carries weights, collective
triggers and the one mid-pass gather (it may stall on collective
completion harmlessly).  Consumers of the early collective's output are
wrapped in tile_wait_until so the Tile scheduler (whose cost model
underestimates collective latency) cannot hoist them ahead of main-pass
work on the same engine.
"""

import sys

sys.path.insert(0, "/opt/trn_rl_repo")

import numpy as np

B, N, D, H, F = 4, 8192, 2048, 512, 64
NCORES = 8
P = 128
EPS = 1e-6
SLOPE = 0.01

_CACHE = {}


def _build(nsh, chunk_tiles=2, xbufs=4, sqbufs=3):
    import concourse.bacc as bacc
    import concourse.mybir as mybir
    from concourse import tile

    dt = mybir.dt.float32
    bt = mybir.dt.bfloat16
    AF = mybir.ActivationFunctionType
    ROWS = B * nsh
    NT = ROWS // P            # 32 row tiles per core
    TPB = nsh // P            # 8 tiles per batch
    KC = D // P               # 16 k-chunks of 128
    JSL = D // NCORES         # 256: L1 output column slice per core
    J2C = JSL // P            # 2:  L1-slice k-chunks for L2
    HC = H // P               # 4:  H chunks of 128
    CT = chunk_tiles
    NCH = NT // CT            # 16 chunks
    CPB = TPB // CT           # 4 chunks per batch
    EPS2 = EPS * float(D) * float(D)   # ss is accumulated scaled by D^2
    assert nsh % (CT * P) == 0

    nc = bacc.Bacc("TRN2", target_bir_lowering=False, debug=False,
                   num_devices=NCORES)

    x = nc.dram_tensor("x", [ROWS, D], dt, kind="ExternalInput")
    w1t = nc.dram_tensor("w1t", [P, KC, JSL], bt, kind="ExternalInput")
    w2t = nc.dram_tensor("w2t", [P, J2C, H], bt, kind="ExternalInput")
    w3t = nc.dram_tensor("w3t", [P, HC, F], bt, kind="ExternalInput")
    b1r = nc.dram_tensor("b1r", [1, JSL], bt, kind="ExternalInput")
    b2rep = nc.dram_tensor("b2rep", [P, HC * B], dt, kind="ExternalInput")
    b3r = nc.dram_tensor("b3r", [1, F], bt, kind="ExternalInput")
    ident = nc.dram_tensor("ident", [B, B], dt, kind="ExternalInput")
    identb = nc.dram_tensor("identb", [B, B], bt, kind="ExternalInput")
    out = nc.dram_tensor("out", [B, F], dt, kind="ExternalOutput")

    groups = [list(range(NCORES))]

    with tile.TileContext(nc) as tc:
        with (
            tc.tile_pool(name="xp", bufs=xbufs) as xp,
            tc.tile_pool(name="sq", bufs=sqbufs) as sq,
            tc.tile_pool(name="sm", bufs=6) as sm,
            tc.tile_pool(name="wp", bufs=1) as wp,
            tc.tile_pool(name="tl", bufs=1) as tl,
            tc.tile_pool(name="pp", bufs=1, space="PSUM") as pp,
            tc.tile_pool(name="dr", bufs=1, space="DRAM") as dr,
        ):
            # constants.  oh1: all-ones column (batch 0 -> psum row 0).
            # oh3[:, 3j:3j+3]: [P,3] slice with column j all-ones
            # (batch 1+j -> psum row j).
            oh1 = wp.tile([P, 1], bt)
            nc.any.memset(oh1[:], 1.0)
            oh3 = wp.tile([P, 9], bt)
            nc.any.memset(oh3[:], 0.0)
            for j in range(3):
                nc.any.memset(oh3[:, 3 * j + j:3 * j + j + 1], 1.0)
            ident4 = wp.tile([B, B], dt)
            nc.gpsimd.dma_start(ident4[:], ident.ap()[:, :])
            ident4b = wp.tile([B, B], bt)
            nc.gpsimd.dma_start(ident4b[:], identb.ap()[:, :])
            ones14 = wp.tile([1, B], bt)
            nc.any.memset(ones14[:], 1.0)

            # weight/bias prefetch on the GpSimd SWDGE ring
            w1sb = wp.tile([P, KC, JSL], bt)
            w2sb = wp.tile([P, J2C, H], bt)
            w3sb = wp.tile([P, HC, F], bt)
            b1row = wp.tile([1, JSL], bt)
            b2T = wp.tile([P, HC * B], dt)
            b3row = wp.tile([1, F], bt)
            nc.gpsimd.dma_start(w1sb[:], w1t.ap()[:, :, :])
            nc.gpsimd.dma_start(w2sb[:], w2t.ap()[:, :, :])
            nc.gpsimd.dma_start(w3sb[:], w3t.ap()[:, :, :])
            nc.gpsimd.dma_start(b1row[:], b1r.ap()[:, :])
            nc.gpsimd.dma_start(b2T[:], b2rep.ap()[:, :])
            nc.gpsimd.dma_start(b3row[:], b3r.ap()[:, :])

            ss_psum = pp.tile([3, D], dt)

            arA_in = dr.tile([1, D], dt)
            arA_out = dr.tile([1, D], dt)
            agB_in = dr.tile([P, 3 * KC], bt)
            agB_out = dr.tile([NCORES * P, 3 * KC], bt)

            ssA_tot = tl.tile([1, D], dt)

            # ---- main pass over x ----
            # 2MB chunks (bandwidth-optimal), except the last 4 tiles go as
            # 1MB chunks: a chunk's completion sem fires only when the whole
            # transfer lands, so 1-tile tail chunks let the final reduces
            # start ~3us earlier instead of serializing after stream end.
            plan, st = [], 0
            for ct in [2] * 14 + [1] * 4:
                plan.append((st, ct))
                st += ct
            assert st == NT
            for st, ct in plan:
                xch = xp.tile([P, CT, D], dt)
                src = x.ap()[st * P:(st + ct) * P, :]
                nc.sync.dma_start(xch[:, :ct, :],
                                  src.rearrange("(p t) d -> p t d", p=P))
                for t in range(ct):
                    g = st + t
                    b = g // TPB
                    xt = xch[:, t, :]
                    negsum = sm.tile([P, 1], dt)
                    nc.vector.reduce_sum(negsum[:], xt, axis=mybir.AxisListType.X,
                                         negate=True)
                    xsq = sq.tile([P, D], bt)
                    nc.scalar.activation(xsq[:], xt, AF.Square,
                                         bias=negsum[:], scale=float(D))
                    if b == 0:
                        lhsT, rows = oh1[:, :], 1
                    else:
                        lhsT, rows = oh3[:, 3 * (b - 1):3 * (b - 1) + 3], 3
                    for q in range(D // 512):
                        nc.tensor.matmul(
                            ss_psum[0:rows, q * 512:(q + 1) * 512],
                            lhsT=lhsT,
                            rhs=xsq[:, q * 512:(q + 1) * 512],
                            start=(g == 0 or g == TPB),
                            stop=(g == TPB - 1 or g == NT - 1))

                if st + ct == TPB:
                    # batch 0 done: drain + AllReduce.  Fired at ~25% of the
                    # stream, this op eats the NCCL barrier + launch-skew +
                    # first-collective cost entirely under the main pass.
                    ssA_sb = tl.tile([1, D], dt)
                    nc.vector.tensor_copy(ssA_sb[:, :D // 2],
                                          ss_psum[0:1, :D // 2])
                    nc.scalar.copy(ssA_sb[:, D // 2:], ss_psum[0:1, D // 2:])
                    nc.scalar.dma_start(arA_in[:], ssA_sb[:])
                    nc.gpsimd.collective_compute(
                        "AllReduce", mybir.AluOpType.add,
                        replica_groups=groups,
                        ins=[arA_in.opt()], outs=[arA_out.opt()])
                    nc.gpsimd.dma_start(ssA_tot[:], arA_out.opt()[:, :])

            # ---- tail: batches 1-3 drain, transpose, AllGather ----
            ssB_sb = tl.tile([3, D], dt)
            nc.vector.tensor_copy(ssB_sb[:, :D // 2], ss_psum[0:3, :D // 2])
            nc.scalar.copy(ssB_sb[:, D // 2:], ss_psum[0:3, D // 2:])
            sstB_psum = pp.tile([P, 3 * KC], dt, tag="sstb")
            for c in range(KC):
                nc.tensor.transpose(sstB_psum[:, c * 3:(c + 1) * 3],
                                    ssB_sb[0:3, c * P:(c + 1) * P],
                                    ident4[0:3, 0:3])
            ssTB = tl.tile([P, 3 * KC], bt)
            nc.vector.tensor_copy(ssTB[:], sstB_psum[:])
            nc.scalar.dma_start(agB_in[:], ssTB[:])
            nc.gpsimd.collective_compute(
                "AllGather", mybir.AluOpType.bypass, replica_groups=groups,
                ins=[agB_in.opt()], outs=[agB_out.opt()])
            gB = tl.tile([P, NCORES, 3 * KC], bt)
            nc.scalar.dma_start(
                gB[:], agB_out.opt().rearrange("(i p) c -> p i c", p=P))

            # assemble ssT [P, KC*B] (c-major, batch minor): batch 0 from
            # the hidden AllReduce (fenced so the scheduler cannot hoist
            # its consumers into the main pass), batches 1-3 from the
            # local 8-way sum of the gathered partials.
            ssT_sb = tl.tile([P, KC * B], dt)
            ssT_v = ssT_sb[:].rearrange("p (c z) -> p c z", z=B)
            with tc.tile_wait_until(0.118):
                sstA_psum = pp.tile([P, KC], dt, tag="ssta")
                for c in range(KC):
                    nc.tensor.transpose(sstA_psum[:, c:c + 1],
                                        ssA_tot[0:1, c * P:(c + 1) * P],
                                        ident4[0:1, 0:1])
                nc.scalar.copy(
                    ssT_v[:, :, 0:1],
                    sstA_psum[:].rearrange("p (c z) -> p c z", z=1))
            redB = tl.tile([P, 3 * KC], dt)
            nc.vector.reduce_sum(redB[:], gB[:].rearrange("p i c -> p c i"),
                                 axis=mybir.AxisListType.X)
            nc.vector.tensor_copy(
                ssT_v[:, :, 1:4],
                redB[:].rearrange("p (c z) -> p c z", z=3))

            # cov = ss/(ss+eps) on the transposed layout
            t1 = tl.tile([P, KC * B], dt)
            nc.vector.tensor_scalar_add(t1[:], ssT_sb[:], EPS2)
            t2 = tl.tile([P, KC * B], dt)
            nc.vector.reciprocal_approx_fast(t2[:], t1[:])
            cov = tl.tile([P, KC * B], bt)
            nc.vector.tensor_mul(cov[:], ssT_sb[:], t2[:])

            # ---- L1: h1 = leaky(cov @ W1[:, slice] + b1[slice])  [B, JSL] ----
            h1_psum = pp.tile([B, JSL], dt, tag="tps", bufs=2)
            for c in range(KC):
                nc.tensor.matmul(h1_psum[:], lhsT=cov[:, c * B:(c + 1) * B],
                                 rhs=w1sb[:, c, :], start=(c == 0), stop=False)
            nc.tensor.matmul(h1_psum[:], lhsT=ones14[:], rhs=b1row[:],
                             start=False, stop=True)
            h1a = tl.tile([B, JSL], dt)
            nc.vector.tensor_scalar_mul(h1a[:], h1_psum[:], SLOPE)
            h1_sb = tl.tile([B, JSL], bt)
            nc.vector.tensor_max(h1_sb[:], h1_psum[:], h1a[:])

            h1T_psum = pp.tile([P, J2C * B], bt, tag="tps", bufs=2)
            for cc in range(J2C):
                nc.tensor.transpose(h1T_psum[:, cc * B:(cc + 1) * B],
                                    h1_sb[0:B, cc * P:(cc + 1) * P], ident4b[:])
            h1T = tl.tile([P, J2C * B], bt)
            nc.vector.tensor_copy(h1T[:], h1T_psum[:])

            # ---- L2 partial: h2p = h1 @ W2[slice, :]  [B, H] ----
            h2_psum = pp.tile([B, H], dt, tag="tps", bufs=2)
            for cc in range(J2C):
                nc.tensor.matmul(h2_psum[:], lhsT=h1T[:, cc * B:(cc + 1) * B],
                                 rhs=w2sb[:, cc, :], start=(cc == 0),
                                 stop=(cc == J2C - 1))
            h2p_sb = tl.tile([B, H], dt)
            nc.vector.tensor_copy(h2p_sb[:, :H // 2], h2_psum[:, :H // 2])
            nc.scalar.copy(h2p_sb[:, H // 2:], h2_psum[:, H // 2:])
            h2T_psum = pp.tile([P, HC * B], dt, tag="tps", bufs=2)
            for r in range(HC):
                nc.tensor.transpose(h2T_psum[:, r * B:(r + 1) * B],
                                    h2p_sb[0:B, r * P:(r + 1) * P], ident4[:])
            h2Tp = tl.tile([P, HC * B], bt)
            nc.vector.tensor_copy(h2Tp[:], h2T_psum[:])

            ag2_in = dr.tile([P, HC * B], bt)
            ag2_out = dr.tile([NCORES * P, HC * B], bt)
            nc.scalar.dma_start(ag2_in[:], h2Tp[:])
            nc.gpsimd.collective_compute(
                "AllGather", mybir.AluOpType.bypass, replica_groups=groups,
                ins=[ag2_in.opt()], outs=[ag2_out.opt()])
            g2 = tl.tile([P, NCORES, HC * B], bt)
            nc.scalar.dma_start(
                g2[:], ag2_out.opt().rearrange("(i p) c -> p i c", p=P))
            h2sum = tl.tile([P, HC * B], dt)
            nc.vector.reduce_sum(h2sum[:], g2[:].rearrange("p i c -> p c i"),
                                 axis=mybir.AxisListType.X)
            h2b = tl.tile([P, HC * B], dt)
            nc.vector.tensor_add(h2b[:], h2sum[:], b2T[:])
            h2a = tl.tile([P, HC * B], dt)
            nc.vector.tensor_scalar_mul(h2a[:], h2b[:], SLOPE)
            h2T = tl.tile([P, HC * B], bt)
            nc.vector.tensor_max(h2T[:], h2b[:], h2a[:])

            # ---- L3: out = h2 @ W3 + b3  [B, F] ----
            out_psum = pp.tile([B, F], dt, tag="tps", bufs=2)
            for r in range(HC):
                nc.tensor.matmul(out_psum[:], lhsT=h2T[:, r * B:(r + 1) * B],
                                 rhs=w3sb[:, r, :], start=(r == 0), stop=False)
            nc.tensor.matmul(out_psum[:], lhsT=ones14[:], rhs=b3row[:],
                             start=False, stop=True)
            out_sb = tl.tile([B, F], dt)
            nc.vector.tensor_copy(out_sb[:], out_psum[:])
            nc.sync.dma_start(out.ap()[:, :], out_sb[:])

    nc.compile()
    return nc


def _get_nc(nsh=N // NCORES):
    key = nsh
    if key not in _CACHE:
        _CACHE[key] = _build(nsh)
    return _CACHE[key]


def _bf(a):
    import ml_dtypes
    return np.ascontiguousarray(a).astype(ml_dtypes.bfloat16)


def make_in_maps(x, W1, b1, W2, b2, W3, b3, nsh=N // NCORES):
    JSL = D // NCORES
    KC, J2C, HC = D // P, JSL // P, H // P
    x = np.asarray(x, dtype=np.float32)
    W1 = np.asarray(W1, dtype=np.float32)
    b1 = np.asarray(b1, dtype=np.float32)
    W2 = np.asarray(W2, dtype=np.float32)
    b2 = np.asarray(b2, dtype=np.float32)
    W3 = np.asarray(W3, dtype=np.float32)
    b3 = np.asarray(b3, dtype=np.float32)
    w3t = _bf(W3.reshape(HC, P, F).transpose(1, 0, 2))
    b2rep = np.ascontiguousarray(
        np.repeat(b2.reshape(HC, P).T, B, axis=1)).astype(np.float32)
    ident = np.eye(B, dtype=np.float32)
    identb = _bf(ident)
    in_maps = []
    for i in range(NCORES):
        xs = np.ascontiguousarray(
            x[:, i * nsh:(i + 1) * nsh, :]).reshape(B * nsh, D)
        w1s = W1[:, i * JSL:(i + 1) * JSL]
        w2s = W2[i * JSL:(i + 1) * JSL, :]
        in_maps.append({
            "x": xs,
            "w1t": _bf(w1s.reshape(KC, P, JSL).transpose(1, 0, 2)),
            "w2t": _bf(w2s.reshape(J2C, P, H).transpose(1, 0, 2)),
            "w3t": w3t,
            "b1r": _bf(b1[i * JSL:(i + 1) * JSL]).reshape(1, JSL),
            "b2rep": b2rep, "b3r": _bf(b3).reshape(1, F),
            "ident": ident, "identb": identb,
        })
    return in_maps


def run(x, W1, b1, W2, b2, W3, b3, nsh=N // NCORES, trace=False):
    from concourse.bass_utils import run_bass_kernel_spmd
    nc = _get_nc(nsh)
    in_maps = make_in_maps(x, W1, b1, W2, b2, W3, b3, nsh=nsh)
    res = run_bass_kernel_spmd(nc, in_maps, list(range(NCORES)), trace=trace)
    return res


def kernel(x, W1, b1, W2, b2, W3, b3):
    res = run(x, W1, b1, W2, b2, W3, b3)
    return np.asarray(res.results[0]["out"], dtype=np.float32)
